# revision 37
# baseline (speedup 1.0000x reference)
"""Trainium2 Bass kernel for the AttZAM attention-weight module.

Computation (full shapes):
    trans_q[b,j,a] = sum_k w_f[j,a,k] * emb_q[b,k]        b=256, j=256, a=128, k=256
    h[b,j,a]      = tanh(trans_q + b_f[j,a])
    g[b,j]        = sum_a h[b,j,a] * w_h[a,0]
    out[b,l]      = sum_j emb_iseq[b,l,j] * g[b,j]        l=1024

Sharding: the j axis (256) is split 8 ways (32 j's per core).  Each core
computes g[b, j_slice] for ALL b, then the partial contraction
sum_{j in slice} emb_iseq[b,l,j] * g[b,j] for all (b,l).  The host sums the
8 partial outputs.  No collectives needed.

Precision: E (emb_iseq) is streamed as fp8 e3m4 (randn data sits in e3m4's
normal range; measured rel-err contribution ~1.35e-2 vs the 2e-2 gate).  The
phase-B matmuls consume the fp8 tiles directly as the moving operand against
a bf16 diag(g) stationary operand.  W/q/h stay bf16; bias fp32.

Per-core kernel:
  Phase A (per j'): matmul lhsT=W_cT[k,ja] bf16, rhs=emb_q.T[k,b] -> psum
  [a=128, b=256]; tanh(+per-partition bias) on ScalarE -> h bf16; N=1 matmuls
  lhsT=h[:,b_chunk], rhs=w_h -> column j' of psum g[b=128, j=32].
  Phase B: PE path (first pe_cnt j's of each group): D = diag(g-col) built
  batched per group ([128, pe_cnt, 128] = ident*g broadcast, split between
  DVE and GpSimd); psum[b=128, l=512] += D.T @ E[j'], accumulated over all
  PE j's in 4 held psum banks.  DVE path (trailing j's of later groups):
  acc[b=128, l=1024] (+)= g-col * E[j'] via fused scalar_tensor_tensor.
  Tail: out = psum + acc (tensor_tensor add on DVE/GpSimd) -> bf16 -> DMA.

Schedule notes (measured on HW):
  - E groups ride the GpSimd SWDGE ring; W rides Sync; the prefetched tail
    group rides Scalar HWDGE at kernel start so the kernel tail after the
    last streamed E tile has no DMA wait.
  - e2 is laid out [128, j', b-chunk, l] so each group's DMA is a single
    8KB-contiguous run per partition (fp8 descriptors stay efficient).
  - Phase A groups lead phase B groups by LEAD in the PE stream.
"""

import sys

import numpy as np
import ml_dtypes

sys.path.insert(0, "/opt/trn_rl_repo")

import concourse.bass as bass  # noqa: E402,F401
import concourse.mybir as mybir  # noqa: E402
import concourse.tile as tile  # noqa: E402
from concourse import bacc  # noqa: E402
from concourse.bass_utils import run_bass_kernel_spmd  # noqa: E402


N_CORES = 8
BSZ, MAX_LEN, D, D_ATTN = 256, 1024, 256, 128
JS = D // N_CORES          # 32 j's per core
JA = JS * D_ATTN           # 4096 rows of the per-core W slice
P = 128                    # partitions
KC = D // P                # 2 k-chunks
NB = BSZ // P              # 2 b-chunks
JG = 4                     # max j's per group
LCH = 512                  # l-chunk (one fp32 psum bank)
NL = MAX_LEN // LCH        # 2 l-chunks

GROUP_SIZES = [2, 2, 4, 8, 8, 8]
assert sum(GROUP_SIZES) == JS
NGRP = len(GROUP_SIZES)
GROUP_STARTS = [sum(GROUP_SIZES[:i]) for i in range(NGRP)]
MAX_G = max(GROUP_SIZES)
LEAD = 2                   # phase-A groups emitted ahead of phase-B groups
N_PRE = 0                  # no prefetch: the 2.1MB tail-group prefetch at
                           # kernel start contended with head/W/E(0) and
                           # stretched their completion latency ~5-8us; the
                           # SWDGE E-ring finishes ~15us before the PE needs
                           # the last group, so streaming it is free.
SYNC_E_GROUPS = set()      # (ring-splitting E measured slower; keep SWDGE only)
# how many TRAILING j's of each group run on DVE instead of PE
OFFLOAD = {2: 1, 3: 2, 4: 3, 5: 2}
PE_CNT = [GROUP_SIZES[i] - OFFLOAD.get(i, 0) for i in range(NGRP)]
PE_JS = [GROUP_STARTS[i] + jj for i in range(NGRP) for jj in range(PE_CNT[i])]
DVE_JS = [
    GROUP_STARTS[i] + jj
    for i in range(NGRP)
    for jj in range(PE_CNT[i], GROUP_SIZES[i])
]
FIRST_PE_J, LAST_PE_J = PE_JS[0], PE_JS[-1]
FIRST_DVE_J = DVE_JS[0] if DVE_JS else None

HEAD_WJ = 8                # j's whose W slice rides in the head DMA (covers
                           # groups 0-2 so phase A never waits on w_mid while
                           # the startup DMA completions are congested)
MID_WJ = 8                 # j's in the w_mid DMA (rest go in w_tail)
# head layout (bf16 cols): q (KC*BSZ) | wh | pad | bias-as-bf16 | w | identity
Q_COLS = KC * BSZ
BIAS_OFF = Q_COLS + 2      # 4-byte aligned
W_OFF = BIAS_OFF + 2 * JS
ID_OFF = W_OFF + KC * HEAD_WJ * D_ATTN
HEADC = ID_OFF + P

BF16 = mybir.dt.bfloat16
F32 = mybir.dt.float32
FP8 = mybir.dt.float8e3
bf16_np = ml_dtypes.bfloat16
fp8_np = ml_dtypes.float8_e3m4

_CACHED_NC = None


def build_nc():
    nc = bacc.Bacc(
        "TRN2",
        target_bir_lowering=False,
        debug=False,
        num_devices=N_CORES,
    )

    head = nc.dram_tensor("head", [P, HEADC], BF16, kind="ExternalInput")
    w_mid = nc.dram_tensor("w_mid", [P, KC, MID_WJ * D_ATTN], BF16, kind="ExternalInput")
    w_tail = nc.dram_tensor(
        "w_tail", [P, KC, (JS - HEAD_WJ - MID_WJ) * D_ATTN], BF16, kind="ExternalInput"
    )
    # e2 holds fp8 bytes but is DECLARED bf16 (half the elem count): the SWDGE
    # ring measured ~250 B/ns with fp8-typed APs vs ~321 B/ns with bf16-typed
    # APs for identical byte counts, so the DMA rides a bf16 AP and compute
    # bitcasts back to fp8.
    e2 = nc.dram_tensor("e2", [P, JS, NB, MAX_LEN // 2], BF16, kind="ExternalInput")
    out = nc.dram_tensor("out", [BSZ, MAX_LEN], BF16, kind="ExternalOutput")

    with tile.TileContext(nc) as tc:
        with (
            tc.tile_pool(name="const", bufs=1) as cpool,
            tc.tile_pool(name="epool", bufs=4) as epool,
            tc.tile_pool(name="hpool", bufs=4) as hpool,
            tc.tile_pool(name="dpool", bufs=6) as dpool,
            tc.tile_pool(name="opool", bufs=2) as opool,
            tc.tile_pool(name="psA", bufs=2, space="PSUM") as psa_pool,
            tc.tile_pool(name="psG", bufs=1, space="PSUM") as psg_pool,
            tc.tile_pool(name="psB", bufs=1, space="PSUM") as psb_pool,
        ):
            head_sb = cpool.tile([P, HEADC], BF16, tag="head", name="head_sb")
            nc.sync.dma_start(out=head_sb, in_=head[:, :])

            wm_sb = cpool.tile([P, KC, MID_WJ * D_ATTN], BF16, tag="wm", name="wm_sb")
            nc.sync.dma_start(out=wm_sb, in_=w_mid[:, :, :])
            wt_sb = cpool.tile(
                [P, KC, (JS - HEAD_WJ - MID_WJ) * D_ATTN], BF16, tag="wt", name="wt_sb"
            )
            # w_tail rides the Scalar HWDGE ring so it lands concurrently with
            # head+w_mid on Sync instead of serializing behind them (phase A
            # for j'>=16 was stalling ~9us on it).
            nc.scalar.dma_start(out=wt_sb, in_=w_tail[:, :, :])

            q_sb = [head_sb[:, kc * BSZ : (kc + 1) * BSZ] for kc in range(KC)]
            wh_sb = head_sb[:, Q_COLS : Q_COLS + 1]
            bias_sb = head_sb[:, BIAS_OFF : BIAS_OFF + 2 * JS].bitcast(F32)

            def w_lhsT(kc, jp):
                if jp < HEAD_WJ:
                    off = W_OFF + kc * HEAD_WJ * D_ATTN + jp * D_ATTN
                    return head_sb[:, off : off + D_ATTN]
                if jp < HEAD_WJ + MID_WJ:
                    off = (jp - HEAD_WJ) * D_ATTN
                    return wm_sb[:, kc, off : off + D_ATTN]
                off = (jp - HEAD_WJ - MID_WJ) * D_ATTN
                return wt_sb[:, kc, off : off + D_ATTN]

            ident = head_sb[:, ID_OFF : ID_OFF + P]

            pre_tiles = {}

            g_sb = cpool.tile([P, NB, JS], F32, tag="g", name="g_sb")
            g_ps = psg_pool.tile([P, NB, JS], F32, tag="gps", name="g_ps")
            acc = [
                cpool.tile([P, MAX_LEN], F32, tag=f"acc{bc}", name=f"acc{bc}")
                for bc in range(NB)
            ]
            ps_out = [
                [
                    psb_pool.tile([P, LCH], F32, tag=f"psB{bc}_{lc}", name=f"psB{bc}_{lc}")
                    for lc in range(NL)
                ]
                for bc in range(NB)
            ]
            etiles = [None] * NGRP
            hq = []            # pending (jp, h) whose g-matmuls are deferred
            dtiles = [None] * NGRP

            def emit_a(i):
                jp0, gsz = GROUP_STARTS[i], GROUP_SIZES[i]
                if i < NGRP - N_PRE:
                    et = epool.tile([P, MAX_G, NB, MAX_LEN // 2], BF16, tag="e", name="et")
                    eng = nc.sync if i in SYNC_E_GROUPS else nc.gpsimd
                    eng.dma_start(
                        out=et[:, :gsz, :, :], in_=e2[:, jp0 : jp0 + gsz, :, :]
                    )
                    etiles[i] = et
                else:
                    etiles[i] = pre_tiles[i]
                for jj in range(gsz):
                    jp = jp0 + jj
                    ps = psa_pool.tile([P, BSZ], F32, tag="psA", name="psA")
                    for kc in range(KC):
                        nc.tensor.matmul(
                            ps,
                            w_lhsT(kc, jp),
                            q_sb[kc],
                            start=(kc == 0),
                            stop=(kc == KC - 1),
                        )
                    h = hpool.tile([P, BSZ], BF16, tag="h", name="h")
                    nc.scalar.activation(
                        h,
                        ps,
                        mybir.ActivationFunctionType.Tanh,
                        bias=bias_sb[:, jp : jp + 1],
                    )
                    hq.append((jp, h))

            def emit_g(i):
                # g-matmuls for group i run one pipeline step after its tanhs,
                # so the PE never stalls waiting on ScalarE.
                jp0, gsz = GROUP_STARTS[i], GROUP_SIZES[i]
                while hq and hq[0][0] < jp0 + gsz:
                    jp, h = hq.pop(0)
                    for bc in range(NB):
                        nc.tensor.matmul(
                            g_ps[:, bc, jp : jp + 1],
                            h[:, bc * P : (bc + 1) * P],
                            wh_sb,
                            start=True,
                            stop=True,
                        )
                nc.vector.tensor_copy(
                    g_sb[:, :, jp0 : jp0 + gsz],
                    g_ps[:, :, jp0 : jp0 + gsz],
                )
                # diag builds for this group's PE j's, one step ahead of use:
                # d[p, jj, c] = ident[p, c] * g[p, jj]
                pe_cnt = PE_CNT[i]
                if pe_cnt > 0:
                    ds = []
                    for bc in range(NB):
                        d = dpool.tile([P, pe_cnt, P], BF16, tag=f"d{bc}", name=f"d{bc}")
                        ib = ident.unsqueeze(1).broadcast_to([P, pe_cnt, P])
                        gb = (
                            g_sb[:, bc, jp0 : jp0 + pe_cnt]
                            .unsqueeze(2)
                            .broadcast_to([P, pe_cnt, P])
                        )
                        nc.vector.tensor_tensor(
                            out=d, in0=ib, in1=gb, op=mybir.AluOpType.mult
                        )
                        ds.append(d)
                    dtiles[i] = ds

            def emit_b(i):
                jp0, gsz = GROUP_STARTS[i], GROUP_SIZES[i]
                pe_cnt = PE_CNT[i]
                et = etiles[i]
                if pe_cnt > 0:
                    ds = dtiles[i]
                    for jj in range(pe_cnt):
                        jp = jp0 + jj
                        for bc in range(NB):
                            for lc in range(NL):
                                nc.tensor.matmul(
                                    ps_out[bc][lc],
                                    ds[bc][:, jj, :],
                                    et[
                                        :, jj, bc, lc * LCH // 2 : (lc + 1) * LCH // 2
                                    ].bitcast(FP8),
                                    start=(jp == FIRST_PE_J),
                                    stop=(jp == LAST_PE_J),
                                )
                for jj in range(pe_cnt, gsz):
                    jp = jp0 + jj
                    for bc in range(NB):
                        if jp == FIRST_DVE_J:
                            nc.vector.tensor_scalar_mul(
                                acc[bc],
                                et[:, jj, bc, :].bitcast(FP8),
                                g_sb[:, bc, jp : jp + 1],
                            )
                        else:
                            nc.vector.scalar_tensor_tensor(
                                out=acc[bc],
                                in0=et[:, jj, bc, :].bitcast(FP8),
                                scalar=g_sb[:, bc, jp : jp + 1],
                                in1=acc[bc],
                                op0=mybir.AluOpType.mult,
                                op1=mybir.AluOpType.add,
                            )

            for i in range(NGRP + LEAD):
                if i < NGRP:
                    emit_a(i)
                if i == 0:
                    for ii in range(NGRP - N_PRE, NGRP):
                        pj0, pg = GROUP_STARTS[ii], GROUP_SIZES[ii]
                        pt = cpool.tile(
                            [P, pg, NB, MAX_LEN // 2], BF16, tag=f"epre{ii}", name=f"epre{ii}"
                        )
                        nc.scalar.dma_start(out=pt, in_=e2[:, pj0 : pj0 + pg, :, :])
                        pre_tiles[ii] = pt
                if 1 <= i <= NGRP:
                    emit_g(i - 1)
                if i >= LEAD:
                    emit_b(i - LEAD)

            # ---- tail: out = psB + acc -> bf16 -> dram, one DMA per b-chunk.
            # ScalarE drains psum (its psum port is fast) while DVE does the
            # adds, so the two tail stages pipeline.
            for bc in range(NB):
                o = opool.tile([P, MAX_LEN], BF16, tag=f"o{bc}", name=f"o{bc}")
                t = opool.tile([P, MAX_LEN], F32, tag=f"t{bc}", name=f"t{bc}")
                for lc in range(NL):
                    nc.scalar.copy(t[:, lc * LCH : (lc + 1) * LCH], ps_out[bc][lc])
                    # t and acc are SBUF, so GpSimd (idle at the tail) can
                    # take half the adds — only psum reads are DVE-only.
                    eng = nc.vector if lc == 0 else nc.gpsimd
                    eng.tensor_tensor(
                        out=o[:, lc * LCH : (lc + 1) * LCH],
                        in0=t[:, lc * LCH : (lc + 1) * LCH],
                        in1=acc[bc][:, lc * LCH : (lc + 1) * LCH],
                        op=mybir.AluOpType.add,
                    )
                nc.sync.dma_start(out=out[bc * P : (bc + 1) * P, :], in_=o)

    nc.compile()
    return nc


def get_nc():
    global _CACHED_NC
    if _CACHED_NC is None:
        _CACHED_NC = build_nc()
    return _CACHED_NC


def make_in_maps(emb_q, emb_iseq, w_f, b_f, w_h):
    """Host-side shard + layout + dtype cast.  Returns list of per-core dicts."""
    q_t = np.ascontiguousarray(emb_q.astype(np.float32).T).astype(bf16_np)  # [k, b]
    qpart = q_t.reshape(KC, P, BSZ).transpose(1, 0, 2).reshape(P, KC * BSZ)
    wh_col = w_h.astype(bf16_np).reshape(1, D_ATTN).T  # [128, 1]
    in_maps = []
    for c in range(N_CORES):
        js, je = c * JS, (c + 1) * JS
        w_slice = w_f[js:je].reshape(JA, D)                       # [ja, k]
        w_t = w_slice.T.astype(bf16_np)                           # [k, ja]
        w2 = np.ascontiguousarray(
            w_t.reshape(KC, P, JA).transpose(1, 0, 2)
        )                                                         # [128, KC, ja]
        bias32 = np.ascontiguousarray(
            b_f[js:je].T.astype(np.float32)
        )                                                         # [a, j'] fp32
        bias_as_bf16 = bias32.view(bf16_np)                       # [128, 64]

        head = np.zeros((P, HEADC), dtype=bf16_np)
        head[:, :Q_COLS] = qpart
        head[:, Q_COLS : Q_COLS + 1] = wh_col
        head[:, BIAS_OFF : BIAS_OFF + 2 * JS] = bias_as_bf16
        head[:, W_OFF:ID_OFF] = w2[:, :, : HEAD_WJ * D_ATTN].reshape(
            P, KC * HEAD_WJ * D_ATTN
        )
        head[:, ID_OFF:] = np.eye(P, dtype=bf16_np)

        w_mid = np.ascontiguousarray(
            w2[:, :, HEAD_WJ * D_ATTN : (HEAD_WJ + MID_WJ) * D_ATTN]
        )
        w_tail = np.ascontiguousarray(w2[:, :, (HEAD_WJ + MID_WJ) * D_ATTN :])

        e_perm = emb_iseq[:, :, js:je].transpose(0, 2, 1)         # [b, j', l]
        e2 = np.ascontiguousarray(
            e_perm.astype(fp8_np)
            .reshape(NB, P, JS, MAX_LEN)
            .transpose(1, 2, 0, 3)
        ).view(bf16_np)                                           # [128, j', NB, l/2] as bf16 bytes
        in_maps.append({"head": head, "w_mid": w_mid, "w_tail": w_tail, "e2": e2})
    return in_maps


def run(in_maps, trace=False, **kwargs):
    nc = get_nc()
    return run_bass_kernel_spmd(
        nc, in_maps, core_ids=list(range(N_CORES)), trace=trace, **kwargs
    )


def kernel(emb_q, emb_iseq, w_f, b_f, w_h):
    emb_q, emb_iseq, w_f, b_f, w_h = (
        np.asarray(x) for x in (emb_q, emb_iseq, w_f, b_f, w_h)
    )
    in_maps = make_in_maps(emb_q, emb_iseq, w_f, b_f, w_h)
    res = run(in_maps, trace=False)
    partial = np.zeros((BSZ, MAX_LEN), dtype=np.float32)
    for r in res.results:
        partial += r["out"].astype(np.float32)
    return partial
